# revision 1
# baseline (speedup 1.0000x reference)
"""Trainium2 Bass kernel for ArmLikenessGNN (GIN message passing + attention pooling).

Strategy (8 NeuronCores, SPMD):
  - Shard by graph: core c owns graphs [8c, 8c+8), their nodes and all edges whose
    dst lies in those graphs. Nodes laid out per-graph-padded so graph slices are
    uniform across cores (static SPMD schedule).
  - Node features kept transposed ("T-layout": [128 feat partitions, node cols]).
  - Per layer: bf16 h-table AllGather (flat row concat of per-core shards) ->
    chunked gpsimd.dma_gather of edge source rows -> segment-sum via one-hot
    matmul accumulated in PSUM -> GIN MLP + LayerNorms computed in T-layout
    (cross-partition reductions via ones-matmuls) -> residual add.
  - Attention pooling + head fully core-local; host concatenates 8x[8] logits.

All edge indexing (sorting, windowing, padding, one-hot matrices, gather index
arrays) is precomputed on host from the concrete inputs; the NEFF is compiled
fresh inside kernel() so every schedule constant is exact.
"""

import sys

sys.path.insert(0, "/opt/trn_rl_repo")

import numpy as np
import ml_dtypes

from concourse import bass, bacc, tile
from concourse import mybir
from concourse.bass_utils import run_bass_kernel_spmd
from concourse.masks import make_identity

HID = 128
N_NODES = 50000
N_EDGES = 640000
N_GRAPHS = 64
N_LAYERS = 3
LN_EPS = 1e-5
NCORES = 8
GPC = N_GRAPHS // NCORES  # graphs per core
CH = 1024  # gather idxs per dma_gather call
# Single SWDGE queue: the dual-queue gather pattern trips DMASW semaphore
# queue-lock violations (seen in CoreSim; suspected cause of a HW hang).
NQUEUES = 1

F32 = mybir.dt.float32
BF16 = mybir.dt.bfloat16
FP8 = mybir.dt.float8e4
AF = mybir.ActivationFunctionType


# ----------------------------------------------------------------------------
# Host-side preprocessing
# ----------------------------------------------------------------------------

def _prep(x, edge_index, batch):
    """Compute the full static layout + per-core data shards."""
    batch = np.asarray(batch).astype(np.int64)
    edge_index = np.asarray(edge_index).astype(np.int64)
    x = np.asarray(x).astype(np.float32)

    # graph boundaries (batch is sorted)
    counts = np.bincount(batch, minlength=N_GRAPHS)
    starts = np.zeros(N_GRAPHS + 1, np.int64)
    np.cumsum(counts, out=starts[1:])
    GP = int(-(-(counts.max() + 1) // 128) * 128)  # graph stride, mult of 128
    NPAD = GPC * GP  # padded nodes per core
    NW = NPAD // 128  # dst windows per core
    SPLIT = 32768  # int16 index ceiling -> two gather sides

    # node -> (core, col)
    g_of = batch
    core_of = (g_of // GPC).astype(np.int64)
    col_of = (g_of % GPC) * GP + (np.arange(N_NODES) - starts[g_of])
    row_of = core_of * NPAD + col_of  # table row

    src, dst = edge_index[0], edge_index[1]
    e_core = core_of[dst]
    e_dcol = col_of[dst]
    e_srow = row_of[src]

    # per (core, window, side) edge lists, sorted by dst col
    cnt = np.zeros((NCORES, NW, 2), np.int64)
    per = [[[None, None] for _ in range(NW)] for _ in range(NCORES)]
    order = np.lexsort((e_srow, e_dcol, e_core))
    e_core_s, e_dcol_s, e_srow_s = e_core[order], e_dcol[order], e_srow[order]
    side_s = (e_srow_s >= SPLIT).astype(np.int64)
    win_s = e_dcol_s // 128
    # split into groups
    key = ((e_core_s * NW + win_s) * 2 + side_s)
    sort2 = np.argsort(key, kind="stable")
    key_sorted = key[sort2]
    grp_starts = np.searchsorted(key_sorted, np.arange(NCORES * NW * 2))
    grp_ends = np.searchsorted(key_sorted, np.arange(NCORES * NW * 2), side="right")
    for c in range(NCORES):
        for w in range(NW):
            for s in range(2):
                k = (c * NW + w) * 2 + s
                sel = sort2[grp_starts[k]:grp_ends[k]]
                per[c][w][s] = (e_srow_s[sel], e_dcol_s[sel] - w * 128)
                cnt[c, w, s] = len(sel)

    # uniform tiles per (window, side)
    T = np.maximum(1, -(-cnt.max(axis=0) // 128))  # [NW, 2] tiles (>=1)
    nta = int(T[:, 0].sum())
    ntb = int(T[:, 1].sum())
    # round each region's tile count so slots are a multiple of CH
    tpc = CH // 128  # tiles per chunk
    nta_pad = -(-nta // tpc) * tpc
    ntb_pad = -(-ntb // tpc) * tpc
    SA, SB = nta_pad * 128, ntb_pad * 128

    # per-core idx + onehot arrays
    idxA = np.zeros((NCORES, SA), np.int32)
    idxB = np.zeros((NCORES, SB), np.int32)
    ohA = np.zeros((NCORES, SA, 128), np.float32)
    ohB = np.zeros((NCORES, SB, 128), np.float32)
    for c in range(NCORES):
        for s, (idx_arr, oh_arr) in ((0, (idxA, ohA)), (1, (idxB, ohB))):
            off = 0
            for w in range(NW):
                rows, dloc = per[c][w][s]
                n = len(rows)
                idx_arr[c, off:off + n] = rows - (SPLIT if s else 0)
                oh_arr[c, off + np.arange(n), dloc] = 1.0
                off += int(T[w, s]) * 128
    del per

    def wrap16(a):  # logical i -> [i%16 rep x8, i//16]
        n = a.shape[-1]
        return np.tile(a.reshape(NCORES, n // 16, 16).transpose(0, 2, 1), (1, 8, 1))

    def wrap128(a):  # [C, S, 128] -> [C, 128, S/128, 128] tile-major
        S = a.shape[1]
        return np.ascontiguousarray(
            a.reshape(NCORES, S // 128, 128, 128).transpose(0, 2, 1, 3))

    idxA_w = wrap16(idxA).astype(np.int16)
    idxB_w = wrap16(idxB).astype(np.int16)
    ohA_w = wrap128(ohA).astype(ml_dtypes.float8_e4m3)
    ohB_w = wrap128(ohB).astype(ml_dtypes.float8_e4m3)

    # per-core x shards in T-layout + graph mask
    xo = np.zeros((NCORES, 13, NPAD), np.float32)
    xa = np.zeros((NCORES, 3, NPAD), np.float32)
    xr = np.zeros((NCORES, 3, NPAD), np.float32)
    gmask = np.full((NCORES, 1, NPAD), -1e30, np.float32)
    others = np.concatenate([x[:, :9], x[:, 15:19]], axis=1)  # [N,13]
    for c in range(NCORES):
        sel = np.nonzero(core_of == c)[0]
        cols = col_of[sel]
        xo[c][:, cols] = others[sel].T
        xa[c][:, cols] = x[sel, 9:12].T
        xr[c][:, cols] = x[sel, 12:15].T
        gmask[c, 0, cols] = 0.0

    layout = dict(GP=GP, NPAD=NPAD, NW=NW, SA=SA, SB=SB, T=T,
                  SPLIT=SPLIT,
                  nta_pad=nta_pad, ntb_pad=ntb_pad)
    shards = dict(xo=xo, xa=xa, xr=xr, gmask=gmask,
                  idxA=idxA_w, idxB=idxB_w, ohA=ohA_w, ohB=ohB_w)
    return layout, shards


def _prep_weights(P):
    """Transpose / fold weights into the layouts the kernel consumes."""
    w = {}

    def f32(a):
        return np.ascontiguousarray(np.asarray(a, np.float32))

    for p in ("axis", "org"):
        W1, b1 = f32(P[p + "_W1"]), f32(P[p + "_b1"])
        g, be = f32(P[p + "_g"]), f32(P[p + "_be"])
        W2, b2 = f32(P[p + "_W2"]), f32(P[p + "_b2"])
        w[p + "W1T"] = f32(W1.T)                       # [3,16]
        w[p + "b1"] = b1[:, None]                      # [16,1]
        w[p + "W2Tf"] = f32((W2 * g[None, :]).T)       # [16,16]
        w[p + "b2f"] = (W2 @ be + b2)[:, None]         # [16,1]
    ninW = f32(P["nin_W"])  # [128,45]
    w["ninWoT"] = f32(ninW[:, :13].T)   # [13,128]
    w["ninWaT"] = f32(ninW[:, 13:29].T)  # [16,128]
    w["ninWrT"] = f32(ninW[:, 29:45].T)  # [16,128]
    w["ninb"] = f32(P["nin_b"])[:, None]  # [128,1]
    for l in range(N_LAYERS):
        W1, b1 = f32(P["conv_W1"][l]), f32(P["conv_b1"][l])
        g, be = f32(P["conv_g"][l]), f32(P["conv_be"][l])
        W2, b2 = f32(P["conv_W2"][l]), f32(P["conv_b2"][l])
        w[f"W1T{l}"] = f32(W1.T)
        w[f"b1{l}"] = b1[:, None]
        w[f"W2Tf{l}"] = f32((W2 * g[None, :]).T)
        w[f"b2f{l}"] = (W2 @ be + b2)[:, None]
        w[f"lng{l}"] = f32(P["ln_g"][l])[:, None]
        w[f"lnb{l}"] = f32(P["ln_b"][l])[:, None]
        w[f"epsv{l}"] = np.full((128, 1), 1.0 + np.float32(P["eps"][l]), np.float32)
    for p, pre in (("gate", "g"), ("head", "h")):
        W1, b1 = f32(P[p + "_W1"]), f32(P[p + "_b1"])
        g, be = f32(P[p + "_g"]), f32(P[p + "_be"])
        W2, b2 = f32(P[p + "_W2"]), f32(P[p + "_b2"])
        w[pre + "W1T"] = f32(W1.T)                     # [128,64]
        w[pre + "b1"] = b1[:, None]                    # [64,1]
        w[pre + "W2Tf"] = f32((W2 * g[None, :]).T)     # [64,1]
        w[pre + "b2f"] = (W2 @ be + b2)[:, None]       # [1,1]
    return w




# column layout for the packed weight/constant tensor (shared host/builder)
def _wpack_layout():
    cols = {}
    off = 0

    def add(name, rows, ncols):
        nonlocal off
        cols[name] = (off, rows, ncols)
        off += ncols

    for p in ("axis", "org"):
        add(p + "W1T", 3, 16)
        add(p + "b1", 16, 1)
        add(p + "W2Tf", 16, 16)
        add(p + "b2f", 16, 1)
    add("ninWoT", 13, 128)
    add("ninWaT", 16, 128)
    add("ninWrT", 16, 128)
    add("ninb", 128, 1)
    for l in range(N_LAYERS):
        add(f"W1T{l}", 128, 128)
        add(f"b1{l}", 128, 1)
        add(f"W2Tf{l}", 128, 128)
        add(f"b2f{l}", 128, 1)
        add(f"lng{l}", 128, 1)
        add(f"lnb{l}", 128, 1)
        add(f"epsv{l}", 128, 1)
    for pre in ("g", "h"):
        add(pre + "W1T", 128, 64)
        add(pre + "b1", 64, 1)
        add(pre + "W2Tf", 64, 1)
        add(pre + "b2f", 1, 1)
    add("ident", 128, 128)
    add("inv_c1", 128, 1)
    add("inv16", 16, 1)
    add("inv64", 64, 1)
    add("ones_1r", 1, 128)
    add("epsc", 1, 1)
    return cols, off


def _pack_weights(P):
    w = _prep_weights(P)
    w["ident"] = np.eye(128, dtype=np.float32)
    w["inv_c1"] = np.full((128, 1), 1.0 / 128.0, np.float32)
    w["inv16"] = np.full((16, 1), 1.0 / 16.0, np.float32)
    w["inv64"] = np.full((64, 1), 1.0 / 64.0, np.float32)
    w["ones_1r"] = np.ones((1, 128), np.float32)
    w["epsc"] = np.full((1, 1), LN_EPS, np.float32)
    cols, total = _wpack_layout()
    pack = np.zeros((128, total), np.float32)
    for name, (off, rows, ncols) in cols.items():
        a = np.asarray(w[name], np.float32)
        assert a.shape == (rows, ncols), (name, a.shape, rows, ncols)
        pack[:rows, off:off + ncols] = a
    return pack


# ----------------------------------------------------------------------------
# Bass kernel builder
# ----------------------------------------------------------------------------

def _lnT(nc, pools, z_out, y, P, ones_c1, ones_1r, eps=None):
    """LayerNorm over the partition dim (P partitions, n cols), T-layout.
    z_out (sbuf) = (y - mean) * rsqrt(var + eps). y is sbuf [P, n].
    Cross-partition sums/broadcasts run as bf16 matmuls (4x PE rate); the
    rounding (~2^-9 relative) is far inside the accuracy budget."""
    n = y.shape[-1]
    psb = pools["psB"]
    pss = pools["psS"]
    wtile = pools["wtile"]
    stile = pools["stile"]
    ones_c1b = pools["ones_c1b"]
    ones_1rb = pools["ones_1rb"]
    yb = wtile(P, n, BF16)
    nc.vector.tensor_copy(yb[:], y[:])
    mu = pss.tile([1, n], F32, tag="psS", name="mu")
    nc.tensor.matmul(mu[:], ones_c1b[:P, :], yb[:])            # [1,n] mean
    mu_sb = stile(n, BF16)
    nc.scalar.activation(mu_sb[:], mu[:], AF.Copy)
    mub = psb.tile([128, n], F32, tag="psB", name="mub")
    nc.tensor.matmul(mub[:P, :], ones_1rb[:1, :P], mu_sb[:])   # bcast [P,n]
    d = wtile(P, n)
    nc.vector.tensor_tensor(out=d[:], in0=y[:], in1=mub[:P, :],
                            op=mybir.AluOpType.subtract)
    sq = wtile(P, n, BF16)
    nc.scalar.activation(sq[:], d[:], AF.Square)
    v = pss.tile([1, n], F32, tag="psS", name="v")
    nc.tensor.matmul(v[:], ones_c1b[:P, :], sq[:])             # [1,n] var
    sd = stile(n)
    nc.scalar.activation(sd[:], v[:], AF.Sqrt, bias=pools["epsc"])
    ri_f = stile(n)
    nc.vector.reciprocal(ri_f[:], sd[:])
    ri = stile(n, BF16)
    nc.vector.tensor_copy(ri[:], ri_f[:])
    rb = psb.tile([128, n], F32, tag="psB", name="rb")
    nc.tensor.matmul(rb[:P, :], ones_1rb[:1, :P], ri[:])       # bcast
    nc.vector.tensor_tensor(out=z_out[:], in0=d[:], in1=rb[:P, :],
                            op=mybir.AluOpType.mult)


def build_kernel(layout):
    GP, NPAD, NW = layout["GP"], layout["NPAD"], layout["NW"]
    SA, SB = layout["SA"], layout["SB"]
    T = layout["T"]
    SPLIT = layout["SPLIT"]
    TPC = CH // 128
    TROWS = NCORES * NPAD
    SLAB = 512
    NSLAB = NPAD // SLAB
    acum = np.concatenate([[0], np.cumsum(T[:, 0])]).astype(int)
    bcum = np.concatenate([[0], np.cumsum(T[:, 1])]).astype(int)

    nc = bacc.Bacc("TRN2", target_bir_lowering=False, debug=False,
                   num_devices=NCORES, dynamic_dma_scratch_size=49152,
                   num_swdge_queues=NQUEUES)

    def param(name, shape, dtype=F32):
        return nc.declare_dram_parameter(name, list(shape), dtype, isOutput=False)

    xo = param("xo", [13, NPAD])
    xa = param("xa", [3, NPAD])
    xr = param("xr", [3, NPAD])
    gmask = param("gmask", [1, NPAD])
    idxA = param("idxA", [128, SA // 16], mybir.dt.int16)
    idxB = param("idxB", [128, SB // 16], mybir.dt.int16)
    ohA = param("ohA", [128, SA // 128, 128], FP8)
    ohB = param("ohB", [128, SB // 128, 128], FP8)
    wcols, wtot = _wpack_layout()
    wpack = param("wpack", [128, wtot])
    out_ext = nc.declare_dram_parameter("out", [1, GPC], F32, isOutput=True)

    with tile.TileContext(nc) as tc:
        with (
            tc.tile_pool(name="const", bufs=1) as constp,
            tc.tile_pool(name="persist", bufs=1) as persist,
            tc.tile_pool(name="work", bufs=6) as work,
            tc.tile_pool(name="wxp", bufs=4) as wxp,
            tc.tile_pool(name="wsp", bufs=4) as wsp,
            tc.tile_pool(name="gatA", bufs=2) as gatA,
            tc.tile_pool(name="gatB", bufs=2) as gatB,
            tc.tile_pool(name="ohpA", bufs=2) as ohpA,
            tc.tile_pool(name="ohpB", bufs=2) as ohpB,
            tc.tile_pool(name="psB", bufs=3, space="PSUM") as psB,
            tc.tile_pool(name="psS", bufs=3, space="PSUM") as psS,
            tc.tile_pool(name="ps128", bufs=2, space="PSUM") as ps128,
            tc.tile_pool(name="dram", bufs=1, space="DRAM") as dram,
        ):
            def wtile(rows, ncols, dt=F32):
                return work.tile([128, 1024], dt, tag="w",
                                 name="wt")[:rows, :ncols]

            def stile(ncols, dt=F32):
                return wsp.tile([1, 1024], dt, tag="ws", name="st")[:, :ncols]

            # packed weights
            wp = constp.tile([128, wtot], F32, name="wp")
            nc.sync.dma_start(out=wp[:], in_=wpack[:])
            W = {}
            for nm, (off, rows, ncols) in wcols.items():
                W[nm] = wp[:rows, off:off + ncols]
            ident = W["ident"]
            inv_c1 = W["inv_c1"]
            inv16 = W["inv16"]
            inv64 = W["inv64"]
            ones_1r = W["ones_1r"]
            onesb = constp.tile([128, 132], BF16, name="onesb")
            nc.vector.memset(onesb[:], 0.0)
            nc.vector.tensor_copy(onesb[:, 0:1], W["inv_c1"])
            nc.vector.tensor_copy(onesb[:16, 1:2], W["inv16"])
            nc.vector.tensor_copy(onesb[:64, 2:3], W["inv64"])
            nc.vector.tensor_copy(onesb[:1, 4:132], W["ones_1r"])
            pools = dict(psB=psB, psS=psS, ps128=ps128, work=work,
                         epsc=W["epsc"], stile=stile, wtile=wtile,
                         ones_1rb=onesb[:1, 4:132])

            idxA_sb = persist.tile([128, SA // 16], mybir.dt.int16, name="idxAs")
            nc.sync.dma_start(out=idxA_sb[:], in_=idxA[:])
            idxB_sb = persist.tile([128, SB // 16], mybir.dt.int16, name="idxBs")
            nc.sync.dma_start(out=idxB_sb[:], in_=idxB[:])
            hT = persist.tile([128, NPAD], F32, name="hT")
            aggT = persist.tile([128, NPAD], F32, name="aggT")
            G = persist.tile([128, GPC], F32, name="G")

            shard_bf = [dram.tile([NPAD, HID], BF16, tag=f"shard{l}",
                                  name=f"shard{l}") for l in range(N_LAYERS)]
            tables = [dram.tile([TROWS, HID], BF16, addr_space="Shared",
                                tag=f"table{l}", name=f"table{l}")
                      for l in range(N_LAYERS)]

            def write_shard(l, s, src_sb):
                for k in range(SLAB // 128):
                    tp = ps128.tile([128, 128], F32, tag="ps128", name="tp")
                    nc.tensor.transpose(tp[:], src_sb[:, k * 128:(k + 1) * 128],
                                        ident)
                    bslab = work.tile([128, 128], BF16, tag="w", name="bs")
                    nc.scalar.activation(bslab[:], tp[:], AF.Copy)
                    r0 = s * SLAB + k * 128
                    nc.sync.dma_start(out=shard_bf[l][r0:r0 + 128, :], in_=bslab[:])

            # =========================== encoder ===========================
            for s in range(NSLAB):
                c0, c1 = s * SLAB, (s + 1) * SLAB
                xo_sb = wxp.tile([13, SLAB], F32, tag="wx", name="xos")
                nc.sync.dma_start(out=xo_sb[:], in_=xo[:, c0:c1])
                xa_sb = wxp.tile([3, SLAB], F32, tag="wx", name="xas")
                nc.sync.dma_start(out=xa_sb[:], in_=xa[:, c0:c1])
                xr_sb = wxp.tile([3, SLAB], F32, tag="wx", name="xrs")
                nc.sync.dma_start(out=xr_sb[:], in_=xr[:, c0:c1])
                zz = {}
                for p, xin in (("axis", xa_sb), ("org", xr_sb)):
                    y1p = psB.tile([128, SLAB], F32, tag="psB", name="y1p")
                    nc.tensor.matmul(y1p[:16, :], W[p + "W1T"], xin[:])
                    y1 = wtile(16, SLAB)
                    nc.scalar.activation(y1[:], y1p[:16, :], AF.Relu,
                                         bias=W[p + "b1"])
                    z1 = wtile(16, SLAB)
                    pools["ones_c1b"] = onesb[:, 1:2]
                    _lnT(nc, pools, z1[:], y1[:], 16, inv16, ones_1r)
                    zp = psB.tile([128, SLAB], F32, tag="psB", name="zp")
                    nc.tensor.matmul(zp[:16, :], W[p + "W2Tf"], z1[:])
                    z2 = wtile(16, SLAB)
                    nc.vector.tensor_scalar_add(z2[:], zp[:16, :], W[p + "b2f"])
                    zz[p] = z2
                h0p = psB.tile([128, SLAB], F32, tag="psB", name="h0p")
                nc.tensor.matmul(h0p[:], W["ninWoT"], xo_sb[:],
                                 start=True, stop=False)
                nc.tensor.matmul(h0p[:], W["ninWaT"], zz["axis"][:],
                                 start=False, stop=False)
                nc.tensor.matmul(h0p[:], W["ninWrT"], zz["org"][:],
                                 start=False, stop=True)
                nc.vector.tensor_scalar_add(hT[:, c0:c1], h0p[:], W["ninb"])
                write_shard(0, s, hT[:, c0:c1])

            # =========================== GIN layers ===========================
            qsel = [0]
            for l in range(N_LAYERS):
                nc.gpsimd.collective_compute(
                    "AllGather", mybir.AluOpType.bypass,
                    replica_groups=[list(range(NCORES))],
                    ins=[shard_bf[l].opt()], outs=[tables[l].opt()],
                )

                side_cfg = (
                    (idxA_sb, ohA, tables[l][0:SPLIT, :], gatA, ohpA),
                    (idxB_sb, ohB, tables[l][SPLIT:TROWS, :], gatB, ohpB),
                )
                chunk_tiles = [{}, {}]

                def get_chunk(s, c):
                    if c not in chunk_tiles[s]:
                        idx_sb, oh_dram, tbl_s, gp, op = side_cfg[s]
                        g = gp.tile([128, TPC, HID], BF16, tag=f"gat{s}",
                                    name=f"g{s}")
                        nc.gpsimd.dma_gather(
                            out_ap=g[:],
                            in_ap=tbl_s[:, :],
                            idxs_ap=idx_sb[:, c * (CH // 16):(c + 1) * (CH // 16)],
                            num_idxs=CH,
                            num_idxs_reg=CH,
                            elem_size=HID,
                            queue_num=qsel[0] % NQUEUES,
                        )
                        qsel[0] += 1
                        oh = op.tile([128, TPC, 128], FP8, tag=f"oh{s}",
                                     name=f"oh{s}")
                        nc.sync.dma_start(
                            out=oh[:],
                            in_=oh_dram[:, c * TPC:(c + 1) * TPC, :])
                        chunk_tiles[s][c] = (g, oh)
                    return chunk_tiles[s][c]

                # window-major consumption: each window accumulates its A then
                # B tiles in one PSUM group; chunks are fetched on demand.
                # MLP slab s follows its 4 windows for tight pipelining.
                for s in range(NSLAB):
                    for w in range(s * (SLAB // 128), (s + 1) * (SLAB // 128)):
                        seq = ([(0, acum[w] + j) for j in range(T[w, 0])]
                               + [(1, bcum[w] + j) for j in range(T[w, 1])])
                        psag = ps128.tile([128, 128], F32, tag="ps128",
                                          name="psag")
                        for i, (sd, gt) in enumerate(seq):
                            g, oh = get_chunk(sd, gt // TPC)
                            t = gt % TPC
                            nc.tensor.matmul(psag[:], g[:, t, :], oh[:, t, :],
                                             start=(i == 0),
                                             stop=(i == len(seq) - 1))
                        nc.vector.tensor_copy(aggT[:, w * 128:(w + 1) * 128],
                                              psag[:])

                    c0, c1 = s * SLAB, (s + 1) * SLAB
                    m = wtile(128, SLAB)
                    nc.vector.scalar_tensor_tensor(
                        out=m[:], in0=hT[:, c0:c1], scalar=W[f"epsv{l}"],
                        in1=aggT[:, c0:c1], op0=mybir.AluOpType.mult,
                        op1=mybir.AluOpType.add)
                    y1p = psB.tile([128, SLAB], F32, tag="psB", name="y1p")
                    nc.tensor.matmul(y1p[:], W[f"W1T{l}"], m[:])
                    y1 = wtile(128, SLAB)
                    nc.scalar.activation(y1[:], y1p[:], AF.Relu, bias=W[f"b1{l}"])
                    z1 = wtile(128, SLAB)
                    pools["ones_c1b"] = onesb[:, 0:1]
                    _lnT(nc, pools, z1[:], y1[:], 128, inv_c1, ones_1r)
                    y2p = psB.tile([128, SLAB], F32, tag="psB", name="y2p")
                    nc.tensor.matmul(y2p[:], W[f"W2Tf{l}"], z1[:])
                    r2 = wtile(128, SLAB)
                    nc.scalar.activation(r2[:], y2p[:], AF.Relu, bias=W[f"b2f{l}"])
                    z2 = wtile(128, SLAB)
                    pools["ones_c1b"] = onesb[:, 0:1]
                    _lnT(nc, pools, z2[:], r2[:], 128, inv_c1, ones_1r)
                    zs = wtile(128, SLAB)
                    nc.vector.tensor_scalar(out=zs[:], in0=z2[:],
                                            scalar1=W[f"lng{l}"],
                                            op0=mybir.AluOpType.mult,
                                            scalar2=W[f"lnb{l}"],
                                            op1=mybir.AluOpType.add)
                    nc.vector.tensor_tensor(out=hT[:, c0:c1], in0=zs[:],
                                            in1=hT[:, c0:c1],
                                            op=mybir.AluOpType.add)
                    if l + 1 < N_LAYERS:
                        write_shard(l + 1, s, hT[:, c0:c1])

            # ===================== pooling + head =====================
            gate_d = dram.tile([1, NPAD], F32, tag="gate_d", name="gate_d")
            for s in range(NSLAB):
                c0, c1 = s * SLAB, (s + 1) * SLAB
                y1p = psB.tile([128, SLAB], F32, tag="psB", name="y1p")
                nc.tensor.matmul(y1p[:64, :], W["gW1T"], hT[:, c0:c1])
                y1 = wtile(64, SLAB)
                nc.scalar.activation(y1[:], y1p[:64, :], AF.Relu, bias=W["gb1"])
                z1 = wtile(64, SLAB)
                pools["ones_c1b"] = onesb[:, 2:3]
                _lnT(nc, pools, z1[:], y1[:], 64, inv64, ones_1r)
                gp = psS.tile([1, SLAB], F32, tag="psS", name="gp")
                nc.tensor.matmul(gp[:], W["gW2Tf"], z1[:])
                gsb = stile(SLAB)
                nc.vector.tensor_scalar_add(gsb[:], gp[:], W["gb2f"])
                gm_sb = stile(SLAB)
                nc.sync.dma_start(out=gm_sb[:], in_=gmask[:, c0:c1])
                gfin = stile(SLAB)
                nc.vector.tensor_tensor(out=gfin[:], in0=gsb[:], in1=gm_sb[:],
                                        op=mybir.AluOpType.add)
                nc.sync.dma_start(out=gate_d[:, c0:c1], in_=gfin[:])

            for gi in range(GPC):
                c0, c1 = gi * GP, (gi + 1) * GP
                gate_sb = stile(GP)
                nc.sync.dma_start(out=gate_sb[:], in_=gate_d[:, c0:c1])
                gmax = stile(1)
                nc.vector.tensor_reduce(out=gmax[:], in_=gate_sb[:],
                                        axis=mybir.AxisListType.X,
                                        op=mybir.AluOpType.max)
                negmax = stile(1)
                nc.vector.tensor_scalar_mul(negmax[:], gmax[:], -1.0)
                e = stile(GP)
                nc.scalar.activation(e[:], gate_sb[:], AF.Exp, bias=negmax[:])
                den = stile(1)
                nc.vector.tensor_reduce(out=den[:], in_=e[:],
                                        axis=mybir.AxisListType.X,
                                        op=mybir.AluOpType.add)
                rden = stile(1)
                nc.vector.reciprocal(rden[:], den[:])
                attn = stile(GP)
                nc.vector.tensor_scalar_mul(attn[:], e[:], rden[:])
                half = GP // 2
                gacc = wtile(128, 2)
                for hf in range(2):
                    h0, h1 = c0 + hf * half, c0 + (hf + 1) * half
                    ab = psB.tile([128, half], F32, tag="psB", name="ab")
                    nc.tensor.matmul(ab[:], ones_1r,
                                     attn[:, hf * half:(hf + 1) * half])
                    wh = wtile(128, half)
                    nc.vector.tensor_tensor(out=wh[:], in0=hT[:, h0:h1],
                                            in1=ab[:], op=mybir.AluOpType.mult)
                    nc.vector.tensor_reduce(out=gacc[:, hf:hf + 1], in_=wh[:],
                                            axis=mybir.AxisListType.X,
                                            op=mybir.AluOpType.add)
                nc.vector.tensor_reduce(out=G[:, gi:gi + 1], in_=gacc[:],
                                        axis=mybir.AxisListType.X,
                                        op=mybir.AluOpType.add)

            y1p = psB.tile([128, GPC], F32, tag="psB", name="y1ph")
            nc.tensor.matmul(y1p[:64, :], W["hW1T"], G[:])
            y1 = wtile(64, GPC)
            nc.scalar.activation(y1[:], y1p[:64, :], AF.Relu, bias=W["hb1"])
            z1 = wtile(64, GPC)
            pools["ones_c1b"] = onesb[:, 2:3]
            _lnT(nc, pools, z1[:], y1[:], 64, inv64, ones_1r)
            lp = psS.tile([1, GPC], F32, tag="psS", name="lp")
            nc.tensor.matmul(lp[:], W["hW2Tf"], z1[:])
            logit = stile(GPC)
            nc.vector.tensor_scalar_add(logit[:], lp[:], W["hb2f"])
            nc.sync.dma_start(out=out_ext[:], in_=logit[:])

    nc.compile()
    return nc


# ----------------------------------------------------------------------------
# Public entry point
# ----------------------------------------------------------------------------

_CACHE = {}


def kernel(**inputs):
    x = inputs["x"]
    edge_index = inputs["edge_index"]
    batch = inputs["batch"]

    layout, shards = _prep(x, edge_index, batch)
    wpack = _pack_weights(inputs)

    key = (layout["GP"], layout["SA"], layout["SB"],
           layout["T"].tobytes())
    nc = _CACHE.get(key)
    if nc is None:
        nc = build_kernel(layout)
        _CACHE[key] = nc

    in_maps = []
    for c in range(NCORES):
        m = {
            "xo": shards["xo"][c], "xa": shards["xa"][c], "xr": shards["xr"][c],
            "gmask": shards["gmask"][c],
            "idxA": shards["idxA"][c], "idxB": shards["idxB"][c],
            "ohA": shards["ohA"][c], "ohB": shards["ohB"][c],
            "wpack": wpack,
        }
        in_maps.append(m)

    import os
    trace = os.environ.get("GNN_KERNEL_TRACE", "1") == "1"
    try:
        res = run_bass_kernel_spmd(nc, in_maps, core_ids=list(range(NCORES)),
                                   trace=trace)
    except Exception:
        if not trace:
            raise
        res = run_bass_kernel_spmd(nc, in_maps, core_ids=list(range(NCORES)),
                                   trace=False)
    global LAST_EXEC_NS
    LAST_EXEC_NS = res.exec_time_ns
    out = np.concatenate([np.asarray(res.results[c]["out"]).reshape(-1)
                          for c in range(NCORES)])
    return out.astype(np.float32)


LAST_EXEC_NS = None


if __name__ == "__main__":
    sys.path.insert(0, "/root/problem")
    import reference
    inp = reference.setup_inputs()
    got = kernel(**{k: np.asarray(v) for k, v in inp.items()})
    exp = np.asarray(reference.reference(**inp))
    err = np.linalg.norm(got - exp) / np.linalg.norm(exp)
    print("Relative error:", err)

